# revision 8
# baseline (speedup 1.0000x reference)
"""GNN message-passing layer (nn_ConvolutionLayer) on 8 Trainium2 NeuronCores.

Math:  out = leakyrelu(diag(1/deg) @ adj @ node @ W^T + b),  deg = adj.sum(-1)

Rewritten for the hardware as
    H1 = [node @ W^T + 1·b^T | 1]          (bias folded: (A(H+1b^T))/deg = AH/deg + b)
    P  = adjT^T @ H1                       (last column of P is deg)
    out = leakyrelu(P[:, :F] * (1/deg))    (leakyrelu is positively homogeneous)

Sharding: data-parallel over batch B=16 -> 2 graphs per core on 8 cores.

Layout/schedule decisions:
  * adj is host-transposed to [m, n] and node to [F, N], both cast to fp8
    e3m4 (measured end-to-end rel err 1.15e-2 vs the 2e-2 gate).  The
    transpose puts the contraction dim on partitions so each 128x128 block is
    directly a matmul stationary operand - no PE transposes - and fp8 halves
    HBM traffic vs bf16.
  * Aggregation uses the fp8 adjT block as stationary and bf16 H1 [128, 129]
    as moving; the 129th H1 column of ones makes deg fall out of the same
    PSUM accumulation group.
  * H1's bias is applied with rank-1 PE matmuls (ones x b_tiled, b=b_hi+b_lo
    exact over bf16 transport) accumulated into the same PSUM group, and the
    PSUM->SBUF bf16 H1 copies run on the Activation engine, keeping DVE free
    so the per-block epilogue reciprocals are never queued behind bulk work.
  * Per graph, blocks nb0-5 accumulate in 6 PSUM banks: an mcl-major
    first-half pass, then an nb-major second-half pass whose epilogues (DVE
    reciprocal + fused ACT Lrelu(scale=1/deg)) pipeline against the PE;
    nb6/nb7 run as full-column tails in the 2 banks that held the H1 builds.
  * The PE p-state ramp is paid down by dep-free warmup matmuls; a dummy
    activation preloads the Lrelu table off the epilogue path.
  * DMA issue is spread across sequencers (~1.2-1.8us per HWDGE DMA of
    issuing-sequencer time, ~1.1us Pool engine per SWDGE descriptor gen):
    W / b ride SP, the per-graph node slices ride ACT, adj slabs + first/last
    stores ride Pool SWDGE, mid stores ride SP.
"""

import ml_dtypes
import numpy as np

import concourse.mybir as mybir
import concourse.tile as tile
from concourse import bacc
from concourse.bass_utils import run_bass_kernel_spmd

B, N, F = 16, 1024, 128
NCORES = 8
G = B // NCORES          # graphs per core
P = 128                  # partitions / tile edge
MC = N // P              # contraction chunks per graph
NB = N // P              # output row blocks per graph
LEAKY_SLOPE = 0.01

W1 = 22                  # warmup matmuls (128 cols each)

f32 = mybir.dt.float32
bf16 = mybir.dt.bfloat16
fp8 = mybir.dt.float8e3

_nc_cache = None


def _build():
    nc = bacc.Bacc("TRN2", target_bir_lowering=False)

    adjt_d = nc.dram_tensor("adjt", [G, N, N], fp8, kind="ExternalInput")
    nodet_d = nc.dram_tensor("nodet", [G, F, N], fp8, kind="ExternalInput")
    auxw_d = nc.dram_tensor("auxw", [P, F], bf16, kind="ExternalInput")
    # auxb: [1, 0:4F] = b_hi tiled 4x; [1, 4F:8F] = b_lo tiled 4x
    auxb_d = nc.dram_tensor("auxb", [1, 8 * F], bf16, kind="ExternalInput")
    out_d = nc.dram_tensor("out", [G, N, F], f32, kind="ExternalOutput")

    with tile.TileContext(nc) as tc:
        with (
            tc.tile_pool(name="const", bufs=1) as const,
            tc.tile_pool(name="rec", bufs=8) as rpool,
            tc.tile_pool(name="ps", bufs=8, space="PSUM") as pspool,
        ):
            # --- input DMAs, issued as early as possible -------------------
            auxw = const.tile([P, F], bf16, tag="auxw")
            nc.sync.dma_start(auxw[:], auxw_d[:])
            auxb = const.tile([1, 8 * F], bf16, tag="auxb")
            nc.sync.dma_start(auxb[:], auxb_d[:])

            nd = [
                const.tile([P, N], fp8, tag=f"nd_{g}", name=f"nd_{g}")
                for g in range(G)
            ]
            for g in range(G):
                nc.scalar.dma_start(nd[g][:], nodet_d[g])

            # g0: two 2-chunk + one 4-chunk slab; g1: four 2-chunk slabs
            at0 = [
                const.tile([P, 2, N], fp8, tag="at0a", name="at0a"),
                const.tile([P, 2, N], fp8, tag="at0b", name="at0b"),
                const.tile([P, 4, N], fp8, tag="at0c", name="at0c"),
            ]
            at1 = [
                const.tile([P, 2, N], fp8, tag=f"at1_{q}", name=f"at1_{q}")
                for q in range(4)
            ]
            for lo, t in ((0, at0[0]), (2, at0[1]), (4, at0[2])):
                k = t.shape[1]
                nc.gpsimd.dma_start(
                    t[:],
                    adjt_d[0, lo * P:(lo + k) * P, :].rearrange(
                        "(mc p) n -> p mc n", p=P
                    ),
                )
            for q in range(4):
                nc.gpsimd.dma_start(
                    at1[q][:],
                    adjt_d[1, q * 2 * P:(q + 1) * 2 * P, :].rearrange(
                        "(mc p) n -> p mc n", p=P
                    ),
                )

            def stat(g, mc, nb):
                """Stationary operand: adjT block [m-part, n] for (g, mc, nb)."""
                if g == 0:
                    t = at0[2][:, mc - 4, :] if mc >= 4 else at0[mc // 2][:, mc % 2, :]
                else:
                    t = at1[mc // 2][:, mc % 2, :]
                return t[:, nb * P:(nb + 1) * P]

            # --- constants / PE+ACT priming --------------------------------
            ones_row = const.tile([1, P], bf16, tag="ones")
            nc.vector.memset(ones_row[:], 1.0)

            # preload the Lrelu table before the real epilogues need it
            act_dummy = const.tile([1, P], f32, tag="actdummy")
            nc.scalar.activation(
                act_dummy[:], ones_row[:], mybir.ActivationFunctionType.Lrelu,
                alpha=LEAKY_SLOPE,
            )

            h1 = [
                const.tile([P, MC, F + 1], bf16, tag=f"h1_{g}", name=f"h1_{g}")
                for g in range(G)
            ]
            for g in range(G):
                nc.vector.memset(h1[g][:, :, F:F + 1], 1.0)

            wps = pspool.tile([P, 512], f32, tag="ps", name="wps")
            for _ in range(W1):
                nc.tensor.matmul(
                    wps[:, 0:P], ones_row[:], ones_row[:], start=True, stop=True
                )

            # --- H1 = [node @ W^T + b | 1], bias via rank-1 matmuls --------
            hps = {}
            for g in range(G):
                for h in range(2):
                    t = pspool.tile([P, 512], f32, tag="ps", name=f"hps_{g}_{h}")
                    hps[g, h] = t
                    for j in range(4):
                        mc = h * 4 + j
                        nc.tensor.matmul(
                            t[:, 0:512].rearrange("p (c f) -> p c f", c=4)[:, j, :],
                            nd[g][:, mc * P:(mc + 1) * P],
                            auxw[:],
                            start=(j == 0),
                            stop=False,
                        )
                    nc.tensor.matmul(
                        t[:, 0:512], ones_row[:], auxb[:, 0:4 * F],
                        start=False, stop=False,
                    )
                    nc.tensor.matmul(
                        t[:, 0:512], ones_row[:], auxb[:, 4 * F:8 * F],
                        start=False, stop=True,
                    )
                    nc.scalar.copy(
                        h1[g][:, h * 4:(h + 1) * 4, 0:F],
                        t[:].rearrange("p (c f) -> p c f", c=4),
                    )

            # --- aggregation ----------------------------------------------
            og = [
                const.tile([P, NB, F], f32, tag=f"og_{g}", name=f"og_{g}")
                for g in range(G)
            ]

            def epilogue(g, nb, ps_nb):
                recip = rpool.tile([P, 1], f32, tag="recip")
                nc.vector.reciprocal(recip[:], ps_nb[:, F:F + 1])
                nc.scalar.activation(
                    og[g][:, nb, :],
                    ps_nb[:, 0:F],
                    mybir.ActivationFunctionType.Lrelu,
                    scale=recip[:],
                    alpha=LEAKY_SLOPE,
                )

            def store(g, lo, hi, engine):
                engine.dma_start(
                    out_d[g, lo * P:(hi + 1) * P, :].rearrange(
                        "(t p) f -> p t f", p=P
                    ),
                    og[g][:, lo:hi + 1, :],
                )

            for g in range(G):
                ps = [
                    pspool.tile([P, 512], f32, tag="ps", name=f"agg_{g}_{nb}")
                    for nb in range(NB)
                ]
                # first-half contraction, mcl-major over nb0-5
                for mcl in range(4):
                    for nb in range(6):
                        nc.tensor.matmul(
                            ps[nb][:, 0:F + 1], stat(g, mcl, nb), h1[g][:, mcl, :],
                            start=(mcl == 0), stop=False,
                        )
                # second-half contraction, nb-major with inline epilogues
                for nb in range(6):
                    for mcl in range(4, 8):
                        nc.tensor.matmul(
                            ps[nb][:, 0:F + 1], stat(g, mcl, nb), h1[g][:, mcl, :],
                            start=False, stop=(mcl == 7),
                        )
                    epilogue(g, nb, ps[nb])
                    if g == 0 and nb == 3:
                        store(g, 0, 3, nc.gpsimd)
                    elif g == 1 and nb in (1, 3, 5):
                        store(g, nb - 1, nb, nc.sync)
                # full-column tail blocks nb6, nb7
                for nb in (6, 7):
                    for mc in range(MC):
                        nc.tensor.matmul(
                            ps[nb][:, 0:F + 1], stat(g, mc, nb), h1[g][:, mc, :],
                            start=(mc == 0), stop=(mc == MC - 1),
                        )
                    epilogue(g, nb, ps[nb])
                store(g, 4, 7, nc.gpsimd) if g == 0 else store(g, 6, 7, nc.gpsimd)

    nc.compile()
    return nc


def _get_nc():
    global _nc_cache
    if _nc_cache is None:
        _nc_cache = _build()
    return _nc_cache


def kernel(node_mat, adj_mat, W, b, _trace=False, _tmpdir=None):
    node_mat = np.asarray(node_mat, dtype=np.float32)
    adj_mat = np.asarray(adj_mat, dtype=np.float32)
    W = np.asarray(W, dtype=np.float32)
    b = np.asarray(b, dtype=np.float32).reshape(F)

    node_t = np.ascontiguousarray(node_mat.transpose(0, 2, 1)).astype(
        ml_dtypes.float8_e3m4
    )  # [B, F, N]
    adj_t = np.ascontiguousarray(adj_mat.transpose(0, 2, 1)).astype(
        ml_dtypes.float8_e3m4
    )  # [B, N(m), N(n)]

    auxw = W.T.astype(ml_dtypes.bfloat16)
    b_hi = b.astype(ml_dtypes.bfloat16)
    b_lo = (b - b_hi.astype(np.float32)).astype(ml_dtypes.bfloat16)
    auxb = np.concatenate([np.tile(b_hi, 4), np.tile(b_lo, 4)]).reshape(1, 8 * F)

    nc = _get_nc()
    in_maps = [
        {
            "adjt": adj_t[c * G:(c + 1) * G],
            "nodet": node_t[c * G:(c + 1) * G],
            "auxw": auxw,
            "auxb": auxb,
        }
        for c in range(NCORES)
    ]
    r = run_bass_kernel_spmd(
        nc, in_maps, core_ids=list(range(NCORES)), trace=_trace, tmpdir=_tmpdir
    )
    out = np.concatenate([r.results[c]["out"] for c in range(NCORES)], axis=0)
    if _trace:
        return out, r
    return out


# revision 9
# speedup vs baseline: 1.0425x; 1.0425x over previous
"""GNN message-passing layer (nn_ConvolutionLayer) on 8 Trainium2 NeuronCores.

Math:  out = leakyrelu(diag(1/deg) @ adj @ node @ W^T + b),  deg = adj.sum(-1)

Rewritten for the hardware as
    H1 = [node @ W^T + 1·b^T | 1]          (bias folded: (A(H+1b^T))/deg = AH/deg + b)
    P  = adjT^T @ H1                       (last column of P is deg)
    out = leakyrelu(P[:, :F] * (1/deg))    (leakyrelu is positively homogeneous)

Sharding: data-parallel over batch B=16 -> 2 graphs per core on 8 cores.

Layout/schedule decisions:
  * adj is host-transposed to [m, n] and node to [F, N], both cast to fp8
    e3m4 (measured end-to-end rel err 1.15e-2 vs the 2e-2 gate).  The
    transpose puts the contraction dim on partitions so each 128x128 block is
    directly a matmul stationary operand - no PE transposes - and fp8 halves
    HBM traffic vs bf16.
  * Aggregation uses the fp8 adjT block as stationary and bf16 H1 [128, 129]
    as moving; the 129th H1 column of ones makes deg fall out of the same
    PSUM accumulation group.
  * Per graph, blocks nb0-5 accumulate in 6 PSUM banks: an mcl-major
    first-half pass, then an nb-major second-half pass whose epilogues (DVE
    reciprocal + fused ACT Lrelu(scale=1/deg)) pipeline against the PE;
    nb6/nb7 run as full-column tails in the 2 banks that held the H1 builds.
    The DVE bias-adds for both graphs complete right before the first
    epilogue reciprocal is needed, so DVE never back-pressures the epilogue.
  * The PE p-state ramp is paid down by dep-free warmup matmuls; a dummy
    activation preloads the Lrelu table (the only ACT table set used).
  * DMA issue is spread across sequencers (~1.2-1.8us of issuing-sequencer
    time per HWDGE DMA, ~1.1us Pool engine per SWDGE descriptor gen):
    aux rides SP, node rides ACT, adj slabs and most stores ride Pool SWDGE,
    and the final single-block store rides the by-then-idle SP.
"""

import ml_dtypes
import numpy as np

import concourse.mybir as mybir
import concourse.tile as tile
from concourse import bacc
from concourse.bass_utils import run_bass_kernel_spmd

B, N, F = 16, 1024, 128
NCORES = 8
G = B // NCORES          # graphs per core
P = 128                  # partitions / tile edge
MC = N // P              # contraction chunks per graph
NB = N // P              # output row blocks per graph
LEAKY_SLOPE = 0.01

W1 = 20                  # warmup matmuls (128 cols each)

f32 = mybir.dt.float32
bf16 = mybir.dt.bfloat16
fp8 = mybir.dt.float8e3

_nc_cache = None


def _build():
    nc = bacc.Bacc("TRN2", target_bir_lowering=False)

    adjt_d = nc.dram_tensor("adjt", [G, N, N], fp8, kind="ExternalInput")
    nodet_d = nc.dram_tensor("nodet", [G, F, N], fp8, kind="ExternalInput")
    # aux: [:, 0:F] = W^T; [0:1, F:2F] = b_hi; [0:1, 2F:3F] = b_lo
    aux_d = nc.dram_tensor("aux", [P, 3 * F], bf16, kind="ExternalInput")
    out_d = nc.dram_tensor("out", [G, N, F], f32, kind="ExternalOutput")

    with tile.TileContext(nc) as tc:
        with (
            tc.tile_pool(name="const", bufs=1) as const,
            tc.tile_pool(name="rec", bufs=8) as rpool,
            tc.tile_pool(name="ps", bufs=8, space="PSUM") as pspool,
        ):
            # --- input DMAs, issued as early as possible -------------------
            aux = const.tile([P, 3 * F], bf16, tag="aux")
            nc.sync.dma_start(aux[:], aux_d[:])
            nd = const.tile([P, G, N], fp8, tag="nd")
            nc.scalar.dma_start(nd[:], nodet_d.rearrange("g f n -> f g n"))

            # g0: two 2-chunk + one 4-chunk slab; g1: one 4-chunk + two 2-chunk
            at0 = [
                const.tile([P, 2, N], fp8, tag="at0a", name="at0a"),
                const.tile([P, 2, N], fp8, tag="at0b", name="at0b"),
                const.tile([P, 4, N], fp8, tag="at0c", name="at0c"),
            ]
            at1 = [
                const.tile([P, 4, N], fp8, tag="at1a", name="at1a"),
                const.tile([P, 2, N], fp8, tag="at1b", name="at1b"),
                const.tile([P, 2, N], fp8, tag="at1c", name="at1c"),
            ]
            for g, tiles in ((0, at0), (1, at1)):
                lo = 0
                for t in tiles:
                    k = t.shape[1]
                    nc.gpsimd.dma_start(
                        t[:],
                        adjt_d[g, lo * P:(lo + k) * P, :].rearrange(
                            "(mc p) n -> p mc n", p=P
                        ),
                    )
                    lo += k

            def stat(g, mc, nb):
                """Stationary operand: adjT block [m-part, n] for (g, mc, nb)."""
                if g == 0:
                    t = at0[2][:, mc - 4, :] if mc >= 4 else at0[mc // 2][:, mc % 2, :]
                else:
                    t = at1[0][:, mc, :] if mc < 4 else at1[1 + (mc - 4) // 2][:, mc % 2, :]
                return t[:, nb * P:(nb + 1) * P]

            # --- constants / PE+ACT priming --------------------------------
            ones_row = const.tile([1, P], bf16, tag="ones")
            nc.vector.memset(ones_row[:], 1.0)

            # preload the Lrelu table before the real epilogues need it
            act_dummy = const.tile([1, P], f32, tag="actdummy")
            nc.scalar.activation(
                act_dummy[:], ones_row[:], mybir.ActivationFunctionType.Lrelu,
                alpha=LEAKY_SLOPE,
            )

            h1 = [
                const.tile([P, MC, F + 1], bf16, tag=f"h1_{g}", name=f"h1_{g}")
                for g in range(G)
            ]
            for g in range(G):
                nc.vector.memset(h1[g][:, :, F:F + 1], 1.0)

            wps = pspool.tile([P, 512], f32, tag="ps", name="wps")
            for _ in range(W1):
                nc.tensor.matmul(
                    wps[:, 0:P], ones_row[:], ones_row[:], start=True, stop=True
                )

            # b broadcast to all 128 partitions, exactly: b_hi + b_lo
            bps = pspool.tile([P, 512], f32, tag="ps", name="bps")
            nc.tensor.matmul(
                bps[:, 0:F], ones_row[:], aux[0:1, F:2 * F], start=True, stop=False
            )
            nc.tensor.matmul(
                bps[:, 0:F], ones_row[:], aux[0:1, 2 * F:3 * F], start=False, stop=True
            )
            b_bc = const.tile([P, F], f32, tag="bbc")
            nc.vector.tensor_copy(b_bc[:], bps[:, 0:F])

            # --- H1 = [node @ W^T + b | 1], both graphs up front -----------
            hps = {}
            for g in range(G):
                for h in range(2):
                    t = pspool.tile([P, 512], f32, tag="ps", name=f"hps_{g}_{h}")
                    hps[g, h] = t
                    for j in range(4):
                        mc = h * 4 + j
                        nc.tensor.matmul(
                            t[:, j * F:(j + 1) * F],
                            nd[:, g, mc * P:(mc + 1) * P],
                            aux[:, 0:F],
                            start=(j == 0),
                            stop=(j == 3),
                        )
            for g in range(G):
                for h in range(2):
                    nc.vector.tensor_add(
                        h1[g][:, h * 4:(h + 1) * 4, 0:F],
                        hps[g, h][:].rearrange("p (c f) -> p c f", c=4),
                        b_bc[:, None, :].to_broadcast((P, 4, F)),
                    )

            # --- aggregation ----------------------------------------------
            og = [
                const.tile([P, NB, F], f32, tag=f"og_{g}", name=f"og_{g}")
                for g in range(G)
            ]

            def epilogue(g, nb, ps_nb):
                recip = rpool.tile([P, 1], f32, tag="recip")
                nc.vector.reciprocal(recip[:], ps_nb[:, F:F + 1])
                nc.scalar.activation(
                    og[g][:, nb, :],
                    ps_nb[:, 0:F],
                    mybir.ActivationFunctionType.Lrelu,
                    scale=recip[:],
                    alpha=LEAKY_SLOPE,
                )

            def store(g, lo, hi, engine):
                engine.dma_start(
                    out_d[g, lo * P:(hi + 1) * P, :].rearrange(
                        "(t p) f -> p t f", p=P
                    ),
                    og[g][:, lo:hi + 1, :],
                )

            for g in range(G):
                ps = [
                    pspool.tile([P, 512], f32, tag="ps", name=f"agg_{g}_{nb}")
                    for nb in range(NB)
                ]
                # first-half contraction, mcl-major over nb0-5
                for mcl in range(4):
                    for nb in range(6):
                        nc.tensor.matmul(
                            ps[nb][:, 0:F + 1], stat(g, mcl, nb), h1[g][:, mcl, :],
                            start=(mcl == 0), stop=False,
                        )
                # second-half contraction, nb-major with inline epilogues
                for nb in range(6):
                    for mcl in range(4, 8):
                        nc.tensor.matmul(
                            ps[nb][:, 0:F + 1], stat(g, mcl, nb), h1[g][:, mcl, :],
                            start=False, stop=(mcl == 7),
                        )
                    epilogue(g, nb, ps[nb])
                    if g == 0 and nb == 3:
                        store(g, 0, 3, nc.gpsimd)
                    elif g == 1 and nb == 2:
                        store(g, 0, 2, nc.gpsimd)
                # full-column tail blocks nb6, nb7
                for nb in (6, 7):
                    for mc in range(MC):
                        nc.tensor.matmul(
                            ps[nb][:, 0:F + 1], stat(g, mc, nb), h1[g][:, mc, :],
                            start=(mc == 0), stop=(mc == MC - 1),
                        )
                    epilogue(g, nb, ps[nb])
                    if g == 1 and nb == 6:
                        store(g, 3, 6, nc.gpsimd)
                if g == 0:
                    store(0, 4, 7, nc.gpsimd)
                else:
                    store(1, 7, 7, nc.sync)

    nc.compile()
    return nc


def _get_nc():
    global _nc_cache
    if _nc_cache is None:
        _nc_cache = _build()
    return _nc_cache


def kernel(node_mat, adj_mat, W, b, _trace=False, _tmpdir=None):
    node_mat = np.asarray(node_mat, dtype=np.float32)
    adj_mat = np.asarray(adj_mat, dtype=np.float32)
    W = np.asarray(W, dtype=np.float32)
    b = np.asarray(b, dtype=np.float32).reshape(F)

    node_t = np.ascontiguousarray(node_mat.transpose(0, 2, 1)).astype(
        ml_dtypes.float8_e3m4
    )  # [B, F, N]
    adj_t = np.ascontiguousarray(adj_mat.transpose(0, 2, 1)).astype(
        ml_dtypes.float8_e3m4
    )  # [B, N(m), N(n)]

    aux = np.zeros((P, 3 * F), dtype=ml_dtypes.bfloat16)
    aux[:, 0:F] = W.T.astype(ml_dtypes.bfloat16)
    b_hi = b.astype(ml_dtypes.bfloat16)
    aux[0, F:2 * F] = b_hi
    aux[0, 2 * F:3 * F] = (b - b_hi.astype(np.float32)).astype(ml_dtypes.bfloat16)

    nc = _get_nc()
    in_maps = [
        {
            "adjt": adj_t[c * G:(c + 1) * G],
            "nodet": node_t[c * G:(c + 1) * G],
            "aux": aux,
        }
        for c in range(NCORES)
    ]
    r = run_bass_kernel_spmd(
        nc, in_maps, core_ids=list(range(NCORES)), trace=_trace, tmpdir=_tmpdir
    )
    out = np.concatenate([r.results[c]["out"] for c in range(NCORES)], axis=0)
    if _trace:
        return out, r
    return out
